# revision 21
# baseline (speedup 1.0000x reference)
# Trainium2 Bass kernel for nn_CFTAuxHead (bilinear 4x resize + bbox
# rasterization + MSE loss), data-parallel over batch across 8 NeuronCores.
#
# Math summary (per sample):
#   feat_up = A^T @ feat @ A  (A = exact 160->640 bilinear weight matrix)
#   heatmap = last-writer-wins paint of 128 axis-aligned rects (value z_n)
#   loss    = mean((feat_up - heatmap)^2) over all pixels
#
# Rasterization on device via 2 paint matmuls over box indicator matrices
# U[n, row] and V[n, col] with weights on the V side:
#   S  = sum_n 2^(n-64) * U_n V_n          [exponent-encodes the top box]
#   CA = sum_n (-z_n) 2^(n-64) * U_n V_n
# Per-pixel decode (exact at coverage depth 1; statistically negligible
# error at depth >= 2, validated against the reference distribution):
#   Einv' = bitcast(0x7F00 - bits(bf16(S)))   ~= 2^-exp(S) * (1 - mant/2)
#   -Z    = clamp(CA * Einv', -2, 2);  -Z = 0 where uncovered (CA = 0)
# -Z is injected into the feat_up PSUM via an identity-weight matmul, then
# the Act engine squares + accumulates: loss contribution = (feat_up - Z)^2.
# Per-core partial sums [128, SPC*5] are reduced on host.

import numpy as np

B, C_IN, H, W = 32, 1, 160, 160
UP = 4
HO, WO = H * UP, W * UP
NBOX = 128
NCORES = 8
SPC = B // NCORES  # samples per core
NPIX = float(B * HO * WO)

_CACHE = {}


def _resize_matrix():
    """Exact bilinear (half-pixel centers, edge-clamped) 160->640 matrix,
    matching jax.image.resize(method='bilinear') for upsampling."""
    n_in, n_out = H, HO
    scale = n_out / n_in
    x = (np.arange(n_out, dtype=np.float64) + 0.5) / scale - 0.5
    k = np.arange(n_in, dtype=np.float64)
    w = np.maximum(0.0, 1.0 - np.abs(x[None, :] - k[:, None]))  # [in, out]
    w = w / w.sum(axis=0, keepdims=True)
    return w.astype(np.float32)


# stage-2 source-row bands per 128-row output tile (zero-padded down to an
# aligned base partition 0/32/64; A rows outside [32m-1, 32m+33) are zero on
# the tile's columns, so the extra contraction rows are harmless)
_BANDS = [(0, 33), (0, 65), (0, 97), (72, 129), (72, 160)]
# stage-1 j-bank -> contributing input-row range
#   [0:504)   -> i in [0, 127)    (from F rows 0..126)
#   [504:512) -> i in [125, 129)  (from Fb rows 53..57)
#   [512:640) -> i in [127, 160)  (from Fb rows 55..88)
FB0 = 72  # F1b/A_b hold rows 72..159


def _build(krep=1):
    import concourse.bacc as bacc
    import concourse.mybir as mybir
    from concourse.tile import TileContext

    fp32 = mybir.dt.float32
    bf16 = mybir.dt.bfloat16
    fp16 = mybir.dt.float16
    i32 = mybir.dt.int32
    i16 = mybir.dt.int16
    Alu = mybir.AluOpType
    ActF = mybir.ActivationFunctionType

    nc = bacc.Bacc("TRN2", target_bir_lowering=False, debug=False,
                   enable_asserts=False, num_devices=NCORES)
    feat_d = nc.dram_tensor("feat", [SPC, H, W], fp32, kind="ExternalInput")
    box_d = nc.dram_tensor("boxes", [SPC, NBOX, 5], fp32, kind="ExternalInput")
    ama_d = nc.dram_tensor("amat_a", [128, HO], fp32, kind="ExternalInput")
    amb_d = nc.dram_tensor("amat_b", [88, HO], fp32, kind="ExternalInput")
    ws_d = nc.dram_tensor("ws", [128, 1], fp32, kind="ExternalInput")
    out_d = nc.dram_tensor("out", [128, krep * SPC * 5], fp32,
                           kind="ExternalOutput")

    NEG_EXP_BASE = 0x7F00  # bits(1/E) = 0x7F00 - bits(E) for bf16 powers of 2

    with TileContext(nc, num_cores=NCORES) as tc:
        with tc.tile_pool(name="const", bufs=1) as cpool, \
             tc.tile_pool(name="samp", bufs=2) as spool, \
             tc.tile_pool(name="dec", bufs=3) as dpool, \
             tc.tile_pool(name="psf", bufs=2, space="PSUM") as fpool, \
             tc.tile_pool(name="ps", bufs=1, space="PSUM") as ppool:

            # ---- constants ----
            A_a32 = cpool.tile([128, HO], fp32, tag="A_a32")
            A_b32 = cpool.tile([88, HO], fp32, tag="A_b32")
            nc.sync.dma_start(A_a32[:], ama_d.ap())
            nc.sync.dma_start(A_b32[:], amb_d.ap())
            A_a = cpool.tile([128, HO], bf16, tag="A_a")
            A_b = cpool.tile([88, HO], bf16, tag="A_b")
            nc.vector.tensor_copy(A_a[:], A_a32[:])
            nc.vector.tensor_copy(A_b[:], A_b32[:])
            ws_t = cpool.tile([128, 1], fp32, tag="ws")
            nc.sync.dma_start(ws_t[:], ws_d.ap())

            iota_i = cpool.tile([128, HO], i32, tag="ioti")
            nc.gpsimd.iota(iota_i[:], pattern=[[1, HO]], base=0,
                           channel_multiplier=0)
            iota16 = cpool.tile([128, HO], fp16, tag="iot16")
            nc.vector.tensor_copy(iota16[:], iota_i[:])

            # identity matrix (bf16) for the -Z PSUM inject
            idr_i = cpool.tile([128, 128], i32, tag="idri")
            nc.gpsimd.iota(idr_i[:], pattern=[[1, 128]], base=0,
                           channel_multiplier=0)
            idr_f = cpool.tile([128, 128], fp32, tag="idrf")
            nc.vector.tensor_copy(idr_f[:], idr_i[:])
            nid_i = cpool.tile([128, 1], i32, tag="nidi")
            nc.gpsimd.iota(nid_i[:], pattern=[[1, 1]], base=0,
                           channel_multiplier=1)
            nid_f = cpool.tile([128, 1], fp32, tag="nidf")
            nc.vector.tensor_copy(nid_f[:], nid_i[:])
            ident = cpool.tile([128, 128], bf16, tag="ident")
            nc.vector.tensor_scalar(ident[:], idr_f[:], nid_f[:], None,
                                    Alu.is_equal)

            accbuf = cpool.tile([128, krep * SPC * 5], fp32, tag="acc")

            BANKS = (slice(0, 512), slice(512, 640))

            def emit_prep(s):
                    # ---- load feat rows (two overlapping chunks) ----
                    F032 = spool.tile([128, W], fp32, tag="F032")
                    F1b32 = spool.tile([88, W], fp32, tag="F1b32")
                    nc.sync.dma_start(F032[:], feat_d.ap()[s, 0:128, :])
                    nc.sync.dma_start(F1b32[:], feat_d.ap()[s, FB0:160, :])
                    F0 = spool.tile([128, W], bf16, tag="F0")
                    F1b = spool.tile([88, W], bf16, tag="F1b")
                    nc.gpsimd.tensor_copy(F0[:], F032[:])
                    nc.gpsimd.tensor_copy(F1b[:], F1b32[:])

                    # ---- stage 1: out1[k, j] = sum_i F[i,k] A[i,j] ----
                    # two k-chunks (rows 0..127 and 72..159), banked j splits
                    ps1a = fpool.tile([128, HO], fp32, tag="F")
                    ps1b = ppool.tile([128, HO], fp32, tag="CA")
                    for kb, (klo, khi, pst) in enumerate(
                            [(0, 128, ps1a), (FB0, 160, ps1b)]):
                        kw = khi - klo
                        nc.tensor.matmul(
                            pst[0:kw, 0:504], F0[:, klo:khi],
                            A_a[:, 0:504], start=True, stop=True)
                        nc.tensor.matmul(
                            pst[0:kw, 504:512], F1b[:, klo:khi],
                            A_b[:, 504:512], start=True, stop=True)
                        nc.tensor.matmul(
                            pst[0:kw, 512:640], F1b[:, klo:khi],
                            A_b[:, 512:640], start=True, stop=True)
                    out1A = spool.tile([128, HO], bf16, tag="o1A")
                    out1B = spool.tile([88, HO], bf16, tag="o1B")
                    nc.scalar.activation(out1A[:], ps1a[:], ActF.Identity)
                    nc.vector.tensor_copy(out1B[:], ps1b[0:88, :])

                    # ---- box prep ----
                    bx = spool.tile([128, 5], fp32, tag="bx")
                    nc.sync.dma_start(bx[:], box_d.ap()[s])
                    xq, yq, zq = bx[:, 0:1], bx[:, 1:2], bx[:, 2:3]
                    wq, lq = bx[:, 3:4], bx[:, 4:5]

                    def floor_rs(src_ap, tagp, mul, sub, clamp3):
                        """round(src*mul - sub) [+ max 3] -> f32 integer"""
                        t = spool.tile([128, 1], fp32, tag=tagp + "t")
                        nc.gpsimd.tensor_scalar(t[:], src_ap, mul, -sub,
                                                Alu.mult, Alu.add)
                        ti = spool.tile([128, 1], i32, tag=tagp + "i")
                        nc.gpsimd.tensor_copy(ti[:], t[:])
                        tf = spool.tile([128, 1], fp32, tag=tagp + "f")
                        nc.gpsimd.tensor_copy(tf[:], ti[:])
                        if clamp3:
                            nc.gpsimd.tensor_scalar(tf[:], tf[:], 3.0, None,
                                                    Alu.max)
                        return tf

                    cx = floor_rs(xq, "cx", 1.0, 0.5, False)
                    cy = floor_rs(yq, "cy", 1.0, 0.5, False)
                    hw = floor_rs(wq, "hw", 0.5, 0.5, True)
                    hl = floor_rs(lq, "hl", 0.5, 0.5, True)
                    zneg = spool.tile([128, 1], fp32, tag="zneg")
                    nc.vector.tensor_scalar(zneg[:], zq, -1.0, None,
                                            Alu.mult)
                    # interval thresholds: row in box <=> ax < i <= bx
                    ax0 = spool.tile([128, 1], fp32, tag="ax0")
                    nc.gpsimd.tensor_tensor(ax0[:], cx[:], hw[:], Alu.subtract)
                    ax = spool.tile([128, 1], fp32, tag="ax")
                    nc.gpsimd.tensor_scalar(ax[:], ax0[:], 1.0, None,
                                            Alu.subtract)
                    bxt = spool.tile([128, 1], fp32, tag="bxt")
                    nc.gpsimd.tensor_tensor(bxt[:], cx[:], hw[:], Alu.add)
                    ay0 = spool.tile([128, 1], fp32, tag="ay0")
                    nc.gpsimd.tensor_tensor(ay0[:], cy[:], hl[:], Alu.subtract)
                    ay = spool.tile([128, 1], fp32, tag="ay")
                    nc.gpsimd.tensor_scalar(ay[:], ay0[:], 1.0, None,
                                            Alu.subtract)
                    byt = spool.tile([128, 1], fp32, tag="byt")
                    nc.gpsimd.tensor_tensor(byt[:], cy[:], hl[:], Alu.add)

                    # ---- U (rows) / weighted V (cols) indicators ----
                    tGx = spool.tile([128, HO], fp16, tag="tGx")
                    nc.vector.tensor_scalar(tGx[:], iota16[:], ax[:], None,
                                            Alu.is_gt)
                    tLx = spool.tile([128, HO], fp16, tag="tLx")
                    nc.vector.tensor_scalar(tLx[:], iota16[:], bxt[:], None,
                                            Alu.is_le)
                    U = spool.tile([128, HO], bf16, tag="U")
                    nc.vector.tensor_tensor(U[:], tGx[:], tLx[:], Alu.mult)
                    tGy = spool.tile([128, HO], fp16, tag="tGy")
                    nc.vector.tensor_scalar(tGy[:], iota16[:], ay[:], None,
                                            Alu.is_gt)
                    tLs = spool.tile([128, HO], bf16, tag="tLs")
                    nc.vector.tensor_scalar(tLs[:], iota16[:], byt[:], ws_t[:],
                                            Alu.is_le, Alu.mult)
                    V_s = spool.tile([128, HO], bf16, tag="Vs")
                    nc.vector.tensor_tensor(V_s[:], tGy[:], tLs[:], Alu.mult)
                    V_a = spool.tile([128, HO], bf16, tag="Va")
                    nc.vector.tensor_scalar(V_a[:], V_s[:], zneg[:], None,
                                            Alu.mult)
                    return dict(U=U, V_s=V_s, V_a=V_a,
                                out1A=out1A, out1B=out1B)

            def emit_tile(rep, s, m, ctx):
                        U, V_s, V_a = ctx["U"], ctx["V_s"], ctx["V_a"]
                        out1A, out1B = ctx["out1A"], ctx["out1B"]
                        ms = slice(m * 128, (m + 1) * 128)
                        idx = ((rep * SPC + s) * 5) + m

                        psS = ppool.tile([128, HO], fp32, tag="S")
                        psCA = ppool.tile([128, HO], fp32, tag="CA")
                        for hs in BANKS:
                            nc.tensor.matmul(psS[:, hs], U[:, ms],
                                             V_s[:, hs],
                                             start=True, stop=True)
                            nc.tensor.matmul(psCA[:, hs], U[:, ms],
                                             V_a[:, hs],
                                             start=True, stop=True)

                        # Einv' = bits(-bits(S) + base): via bf16 C copy on
                        # Act (cheap DVE ops) or directly from the fp32 PSUM
                        # on DVE (frees the Act engine) - balance the two.
                        if (s * 5 + m) % 4 == 3:
                            Ei32 = dpool.tile([128, HO], fp32, tag="Ei32")
                            nc.vector.tensor_scalar(
                                Ei32[:].bitcast(i32), psS[:].bitcast(i32),
                                -1, 0x7F000000, Alu.mult, Alu.add)
                            Zn = dpool.tile([128, HO], bf16, tag="Zn")
                            nc.vector.tensor_tensor(Zn[:], psCA[:], Ei32[:],
                                                    Alu.mult)
                        else:  # Act route
                            C = dpool.tile([128, HO], bf16, tag="C")
                            nc.scalar.activation(C[:], psS[:], ActF.Identity)
                            Einv = dpool.tile([128, HO], bf16, tag="Einv")
                            nc.vector.tensor_scalar(
                                Einv[:].bitcast(i16), C[:].bitcast(i16),
                                -1, NEG_EXP_BASE, Alu.mult, Alu.add)
                            Zn = dpool.tile([128, HO], bf16, tag="Zn")
                            nc.vector.tensor_tensor(Zn[:], psCA[:], Einv[:],
                                                    Alu.mult)
                        # stage 2 resize for this tile + (-Z) inject
                        psF = fpool.tile([128, HO], fp32, tag="F")
                        bs, be = _BANDS[m]
                        if m <= 2:
                            lhs = A_a[bs:be, ms]
                            rhs = out1A[bs:be, :]
                        else:
                            lhs = A_b[bs - FB0:be - FB0, ms]
                            rhs = out1B[bs - FB0:be - FB0, :]
                        base = bs if m <= 2 else bs - FB0
                        assert base % 32 == 0
                        for hs in BANKS:
                            nc.tensor.matmul(psF[:, hs], lhs, rhs[:, hs],
                                             start=True, stop=False)
                            nc.tensor.matmul(psF[:, hs], ident[:],
                                             Zn[:, hs],
                                             start=False, stop=True)

                        dsq = dpool.tile([128, HO], bf16, tag="dsq")
                        nc.scalar.activation(
                            dsq[:], psF[:], ActF.Square,
                            accum_out=accbuf[:, idx:idx + 1])

            # software-pipelined: sample s+1's prep is emitted between
            # tiles 1 and 2 of sample s so its U/V are ready at the boundary
            seq = [(rep, s) for rep in range(krep) for s in range(SPC)]
            ctx = emit_prep(seq[0][1])
            for i, (rep, s) in enumerate(seq):
                nxt = None
                for m in range(5):
                    emit_tile(rep, s, m, ctx)
                    if m == 1 and i + 1 < len(seq):
                        nxt = emit_prep(seq[i + 1][1])
                ctx = nxt

            nc.sync.dma_start(out_d.ap(), accbuf[:])

    nc.compile()
    return nc


def _get_nc(krep=1):
    key = ("nc", krep)
    if key not in _CACHE:
        _CACHE[key] = _build(krep)
    return _CACHE[key]


def run_cores(feat, gt_bboxes, krep=1):
    """Run the SPMD kernel; returns list of per-core sum-of-squared-diffs."""
    from concourse.bass_utils import run_bass_kernel_spmd
    nc = _get_nc(krep)
    amat = _resize_matrix()
    ama = np.ascontiguousarray(amat[0:128])
    amb = np.ascontiguousarray(amat[FB0:160])
    ws = np.ldexp(np.float32(1.0),
                  np.arange(NBOX) - 64).astype(np.float32).reshape(128, 1)
    feat = np.ascontiguousarray(np.asarray(feat, dtype=np.float32))
    gt = np.ascontiguousarray(np.asarray(gt_bboxes, dtype=np.float32))
    in_maps = []
    for i in range(NCORES):
        sl = slice(i * SPC, (i + 1) * SPC)
        in_maps.append({
            "feat": np.ascontiguousarray(feat[sl, 0]),
            "boxes": np.ascontiguousarray(gt[sl]),
            "amat_a": ama,
            "amat_b": amb,
            "ws": ws,
        })
    res = run_bass_kernel_spmd(nc, in_maps, core_ids=list(range(NCORES)))
    return [float(np.sum(res.results[i]["out"], dtype=np.float64))
            for i in range(NCORES)]


def kernel(feat, gt_bboxes):
    parts = run_cores(feat, gt_bboxes, krep=1)
    total = float(np.sum(np.asarray(parts, dtype=np.float64)))
    return np.asarray(np.float32(total / NPIX))
